# revision 1
# baseline (speedup 1.0000x reference)
"""Trainium2 Bass kernel for nn_Model4 (retrieval_knn) — v5.

Sharding: pure 8-way row sharding. Core c owns rows R_c = [128c, 128c+128).
Every MHA is flash-style: each core projects K/V only for its own 128 rows
(s-shard), computes exp-score and context partials for ALL 1024 query rows
against that shard, and one 8-rank ReduceScatter(add) sums the partials and
hands core c the context for its own rows (~0.26 MB bf16 => ~22 us).

Fusions (host-side weight algebra):
- qp_tl / qp_tg / qp_rt computed straight from text via W_q @ W_text fused
  weights (all rows, redundant per core - cheaper than gathering).
- qp_tl / qp_tg run on RAW text; the l2-norm column scale is applied to the
  GEMM output (postscale), so they start before the norm stats finish.
- gt and ff are never materialized: their out-projections are folded into
  the NEXT MHA's K/V projections (kp_ff = ctxn_tg @ (Wk_ff Wo_tg)^T +
  text-part, etc.), shortening every ReduceScatter -> next-attention hop.
- qp_ff and kp_ff ride fp8 (host pre-scales by 32, exp scale divides it
  back out); the qp_ff AllGather is the only gather left (1 MB fp8, 41 us).

The final cosine/logits stage is fully local: each core emits
logitsT [256 group-cols, 128 own-rows]; the host assembles [4,256,256].
"""
import sys

sys.path.insert(0, "/opt/trn_rl_repo")

import ml_dtypes
import numpy as np

import concourse.bass as bass  # noqa: F401
import concourse.tile as tile
import concourse.mybir as mybir
from concourse import bacc
from concourse.bass_utils import run_bass_kernel_spmd

E = 1024
P = 128
KO = E // P            # 8 feature chunks
MR = 128               # rows owned per core
NCORES = 8
F32 = mybir.dt.float32
F32R = mybir.dt.float32r
BF16 = mybir.dt.bfloat16
FP8 = mybir.dt.float8e4
AF = mybir.ActivationFunctionType
ALLR = [[0, 1, 2, 3, 4, 5, 6, 7]]
EPS = 1e-8
CTX_ELEMS = P * KO * MR            # 131072 ctx elems per l-chunk
DEN_ELEMS = 4 * MR                 # 512 denominator elems per l-chunk
CHUNK = CTX_ELEMS + DEN_ELEMS      # RS chunk per rank
FSC = 32.0                         # fp8 pre-scale for the ff q/k path

_CACHE = {}


def build_nc():
    nc = bacc.Bacc("TRN2", target_bir_lowering=False, debug=False,
                   num_devices=NCORES)
    dram = {}

    def din(name, shape, dt=BF16):
        dram[name] = nc.dram_tensor(name, shape, dt, kind="ExternalInput").ap()

    # features (feat-major): full text + this core's slices
    din("xt_full", [E, E])
    din("xt_my", [E, MR])
    din("xl_my", [E, MR])
    din("xg_my", [E, MR])
    din("xlg", [E, 256])
    # weights, host-transposed to [in, out], bf16
    for w in ("wfq_tl", "wfq_tg", "wfq_rt",      # fused text->qp
              "w_tl", "w_rep", "wq_ff",
              "wak_ff", "wbk_ff", "wav_ff", "wbv_ff",   # gt-fused ff K/V
              "wak_rt", "wav_rt", "wk_rt", "wv_rt",     # ff-fused rt K/V
              "wk_tl", "wv_tl", "wo_tl",
              "wk_tg", "wv_tg", "wo_rt"):
        din(w, [E, E])
    for b in ("bfq_tl", "bfq_tg", "bfq_rt", "b_tl", "b_rep", "bq_ff",
              "pos_l", "pos_g",
              "bk_tl", "bv_tl", "bo_tl", "bk_tg", "bv_tg",
              "bfk_ff", "bvh_ff", "bkt_rt", "bvh_rt", "bo_rt"):
        din(b, [E], F32)

    out_logits = nc.dram_tensor("logits", [256, MR], F32,
                                kind="ExternalOutput").ap()

    from contextlib import ExitStack
    with tile.TileContext(nc) as tc, ExitStack() as ctx:
        consts = ctx.enter_context(tc.tile_pool(name="consts", bufs=1))
        weights = ctx.enter_context(tc.tile_pool(name="weights", bufs=2))
        qps = ctx.enter_context(tc.tile_pool(name="qps", bufs=2))
        bigs = ctx.enter_context(tc.tile_pool(name="bigs", bufs=1))
        exps = ctx.enter_context(tc.tile_pool(name="exps", bufs=1))
        ctxs = ctx.enter_context(tc.tile_pool(name="ctxs", bufs=1))
        sqs = ctx.enter_context(tc.tile_pool(name="sqs", bufs=1))
        smalls = ctx.enter_context(tc.tile_pool(name="smalls", bufs=2))
        kvs = ctx.enter_context(tc.tile_pool(name="kvs", bufs=2))
        acts = ctx.enter_context(tc.tile_pool(name="acts", bufs=1))
        bcs = ctx.enter_context(tc.tile_pool(name="bcs", bufs=1))
        outs = ctx.enter_context(tc.tile_pool(name="outs", bufs=1))
        psA = ctx.enter_context(tc.tile_pool(name="psA", bufs=3, space="PSUM"))
        psB = ctx.enter_context(tc.tile_pool(name="psB", bufs=3, space="PSUM"))
        psD = ctx.enter_context(tc.tile_pool(name="psD", bufs=2, space="PSUM"))
        dram_p = ctx.enter_context(tc.tile_pool(name="dram_p", bufs=1,
                                                space="DRAM"))

        # ---------- constants ----------
        ones_cb = consts.tile([P, 1], BF16)
        nc.vector.memset(ones_cb, 1.0)
        ones_cf32 = consts.tile([P, 1], F32)
        nc.vector.memset(ones_cf32, 1.0)
        ones_col = consts.tile([P, 1], F32R)
        nc.vector.tensor_copy(ones_col, ones_cf32)
        ones_rf = consts.tile([1, P], F32)
        nc.vector.memset(ones_rf, 1.0)
        ones_row = consts.tile([1, P], F32R)
        nc.vector.tensor_copy(ones_row, ones_rf)
        ones_row_b = consts.tile([1, P], BF16)
        nc.vector.tensor_copy(ones_row_b, ones_rf)

        def load_bias_pp(name):
            t = consts.tile([P, KO], F32, name=f"c_{name}")
            nc.sync.dma_start(t, dram[name].rearrange("(c p) -> p c", p=P))
            return t

        bias_pp = {}

        # ---------- helpers ----------
        def load_w(name):
            t = weights.tile([P, KO, E], BF16, tag="w", name=f"w_{name}")
            nc.sync.dma_start(t, dram[name].rearrange("(ko p) c -> p ko c",
                                                      p=P))
            return t

        def gemm(w, act, out, bias=None, residual=None, postscale=None):
            """out[:, c, :] = sum_ko w[:, ko, cP:(c+1)P].T @ act[:, ko, :].

            postscale [128, R]: out = (mm result) * postscale + bias (folds
            the l2-norm column scale so the GEMM can run on raw input)."""
            C = out.shape[1]
            R = act.shape[2]
            LH = 512 if R > 512 else R
            for c in range(C):
                for lh in range(0, R, LH):
                    ps = psA.tile([P, LH], F32, tag="mm", name="ps_g")
                    for ko in range(KO):
                        nc.tensor.matmul(ps, w[:, ko, c * P:(c + 1) * P],
                                         act[:, ko, lh:lh + LH],
                                         start=(ko == 0), stop=(ko == KO - 1))
                    o = out[:, c, lh:lh + LH]
                    if postscale is not None:
                        nc.any.tensor_mul(o, ps, postscale[:, lh:lh + LH])
                        if bias is not None:
                            nc.any.tensor_scalar_add(o, o, bias[:, c:c + 1])
                    elif bias is not None:
                        nc.any.tensor_scalar_add(o, ps, bias[:, c:c + 1])
                    else:
                        nc.any.tensor_copy(out=o, in_=ps)
                    if residual is not None:
                        nc.any.tensor_add(o, o, residual[:, c, lh:lh + LH])

        def gemm_sm(w, act, out):
            """s-major GEMM: out [128 s, 2, 512] = act(lhsT) @ w, no bias."""
            for dh in range(2):
                ps = psA.tile([P, 512], F32, tag="mm", name="ps_sm")
                for ko in range(KO):
                    nc.tensor.matmul(ps, act[:, ko],
                                     w[:, ko, dh * 512:(dh + 1) * 512],
                                     start=(ko == 0), stop=(ko == KO - 1))
                nc.any.tensor_copy(out=out[:, dh], in_=ps)
            return out

        def bcast_row(row, n, dt=F32):
            """[1, n] row -> [128, n] broadcast via K=1 matmul."""
            out = bcs.tile([P, n], dt, tag=f"bc{n}", name="bc")
            ones = ones_row_b if row.dtype == BF16 else ones_row
            LH = 512 if n > 512 else n
            for lh in range(0, n, LH):
                ps = psB.tile([P, LH], F32, tag="mmb", name="ps_bc")
                nc.tensor.matmul(ps, ones, row[0:1, lh:lh + LH],
                                 start=True, stop=True)
                nc.any.tensor_copy(out=out[:, lh:lh + LH], in_=ps)
            return out

        def stats_inv(src, sqdt=BF16, with_eps=False):
            """src [128, KO, R] -> inv row [1, R] (1/||col||)."""
            R = src.shape[2]
            LH = 512 if R > 512 else R
            nlh = R // LH
            pss = [psD.tile([1, LH], F32, tag="cs", name=f"ps_cs{i}")
                   for i in range(nlh)]
            ones = ones_cb if sqdt == BF16 else ones_col
            for ko in range(KO):
                sq = sqs.tile([P, R], sqdt, tag=f"sq{R}{sqdt}", name="sq")
                nc.any.tensor_mul(sq, src[:, ko], src[:, ko])
                for i in range(nlh):
                    nc.tensor.matmul(pss[i], ones, sq[:, i * LH:(i + 1) * LH],
                                     start=(ko == 0), stop=(ko == KO - 1))
            nbufs = 1 if R > 512 else 2
            norm = smalls.tile([1, R], F32, tag=f"nrm{R}", bufs=nbufs,
                               name="nrm")
            for i in range(nlh):
                nc.scalar.sqrt(norm[0:1, i * LH:(i + 1) * LH], pss[i])
            if with_eps:
                nc.vector.tensor_scalar_max(norm, norm, EPS)
            inv = smalls.tile([1, R], BF16, tag=f"inv{R}", bufs=nbufs,
                              name="inv")
            with nc.allow_low_precision(reason="bf16 norm scale intended"):
                nc.vector.reciprocal(inv, norm)
            return inv

        def normalize(src, out, pos=None):
            R = src.shape[2]
            inv = stats_inv(src)
            bc = bcast_row(inv, R, dt=BF16)
            for ko in range(KO):
                nc.any.tensor_mul(out[:, ko], src[:, ko], bc)
                if pos is not None:
                    nc.vector.tensor_scalar_add(out[:, ko], out[:, ko],
                                                pos[:, ko:ko + 1])

        def kv_project(m, src, kpdt=BF16):
            """kp [128, 8, 128] feat-major (+bias); vp [128, 2, 512] s-major."""
            wk = load_w(f"wk_{m}")
            kp = kvs.tile([P, KO, MR], kpdt, tag="kp", name=f"kp_{m}")
            gemm(wk, src, kp, bias=bias_pp[f"bk_{m}"])
            wv = load_w(f"wv_{m}")
            vp = kvs.tile([P, 2, 512], BF16, tag="vp", name=f"vp_{m}")
            gemm_sm(wv, src, vp)
            return kp, vp

        def kv_fused(m, ctxn, ktext, vtext, kpdt=BF16):
            """kp = ctxn @ A_k + ktext; vp = ctxn @ A_v + vtext."""
            wak = load_w(f"wak_{m}")
            kp = kvs.tile([P, KO, MR], kpdt, tag="kp", name=f"kp_{m}")
            for c in range(KO):
                ps = psA.tile([P, MR], F32, tag="mm", name="ps_kf")
                for ko in range(KO):
                    nc.tensor.matmul(ps, wak[:, ko, c * P:(c + 1) * P],
                                     ctxn[:, ko], start=(ko == 0),
                                     stop=(ko == KO - 1))
                nc.any.tensor_add(kp[:, c], ps, ktext[:, c])
            wav = load_w(f"wav_{m}")
            vp = kvs.tile([P, 2, 512], BF16, tag="vp", name=f"vp_{m}")
            for dh in range(2):
                ps = psA.tile([P, 512], F32, tag="mm", name="ps_vf")
                for ko in range(KO):
                    nc.tensor.matmul(ps, ctxn[:, ko],
                                     wav[:, ko, dh * 512:(dh + 1) * 512],
                                     start=(ko == 0), stop=(ko == KO - 1))
                nc.any.tensor_add(vp[:, dh], ps, vtext[:, dh])
            return kp, vp

        def attention_partial(m, kp, vp, qp, rs_in, sc=0.0625):
            """Flash partials vs this core's s-shard; pack ctx+den to rs_in."""
            expT = exps.tile([P, 4, E], BF16, tag="exp", name=f"exp_{m}")
            for h in range(4):
                for lh in range(2):
                    ps = psA.tile([P, 512], F32, tag="mm", name="ps_sc")
                    for dk in range(2):
                        nc.tensor.matmul(
                            ps, kp[:, 2 * h + dk],
                            qp[:, 2 * h + dk, lh * 512:(lh + 1) * 512],
                            start=(dk == 0), stop=(dk == 1))
                    nc.scalar.activation(expT[:, h, lh * 512:(lh + 1) * 512],
                                         ps, AF.Exp, scale=sc)
            den = smalls.tile([1, 4, E], BF16, tag="den", bufs=1,
                              name=f"den_{m}")
            for h in range(4):
                for lh in range(2):
                    psd = psD.tile([1, 512], F32, tag="cs", name="ps_den")
                    nc.tensor.matmul(psd, ones_cb,
                                     expT[:, h, lh * 512:(lh + 1) * 512],
                                     start=True, stop=True)
                    nc.any.tensor_copy(
                        out=den[0:1, h, lh * 512:(lh + 1) * 512], in_=psd)
            ctxp = ctxs.tile([P, KO, E], BF16, tag="ctx", name=f"ctx_{m}")
            for dvc in range(KO):
                h = dvc // 2
                for lh in range(2):
                    ps = psA.tile([P, 512], F32, tag="mm", name="ps_cx")
                    nc.tensor.matmul(
                        ps,
                        vp[:, dvc // 4, (dvc % 4) * P:(dvc % 4 + 1) * P],
                        expT[:, h, lh * 512:(lh + 1) * 512],
                        start=True, stop=True)
                    nc.any.tensor_copy(out=ctxp[:, dvc, lh * 512:(lh + 1) * 512],
                                       in_=ps)
            for lc in range(8):
                off = lc * CHUNK
                nc.sync.dma_start(
                    rs_in[off:off + CTX_ELEMS].rearrange(
                        "(p a b) -> p a b", p=P, a=KO),
                    ctxp[:, :, lc * MR:(lc + 1) * MR])
                nc.sync.dma_start(
                    rs_in[off + CTX_ELEMS:off + CHUNK].rearrange(
                        "(o a b) -> o a b", o=1, a=4),
                    den[0:1, :, lc * MR:(lc + 1) * MR])

        def reduce_scatter(m):
            rs_in = dram_p.tile([8 * CHUNK], BF16, name=f"rsin_{m}")
            rs_out = dram_p.tile([CHUNK], BF16, name=f"rsout_{m}")
            return rs_in, rs_out

        def rs_launch(rs_in, rs_out):
            nc.gpsimd.collective_compute(
                "ReduceScatter", mybir.AluOpType.add,
                replica_groups=ALLR,
                ins=[rs_in.opt()], outs=[rs_out.opt()])

        def ctx_normalize(m, rs_out, bvname):
            """Unpack own ctx/den; return ctxn = ctx/den + bv (pre-out-proj)."""
            ctx_my = acts.tile([P, KO, MR], BF16, tag="cmy", name=f"cmy_{m}")
            nc.sync.dma_start(ctx_my, rs_out[0:CTX_ELEMS].rearrange(
                "(p a b) -> p a b", p=P, a=KO))
            den_my = smalls.tile([1, 4, MR], BF16, tag="dmy", bufs=1,
                                 name=f"dmy_{m}")
            nc.sync.dma_start(den_my, rs_out[CTX_ELEMS:CHUNK].rearrange(
                "(o a b) -> o a b", o=1, a=4))
            rec = smalls.tile([1, 4, MR], BF16, tag="rec", bufs=1,
                              name=f"rec_{m}")
            with nc.allow_low_precision(reason="bf16 softmax denom"):
                nc.vector.reciprocal(rec, den_my)
            bc4 = bcs.tile([P, 4, MR], BF16, tag="bc4", bufs=1,
                           name=f"bc4_{m}")
            for h in range(4):
                ps = psB.tile([P, MR], F32, tag="mmb", name="ps_b4")
                nc.tensor.matmul(ps, ones_row_b, rec[0:1, h], start=True,
                                 stop=True)
                nc.any.tensor_copy(out=bc4[:, h], in_=ps)
            ctxn = acts.tile([P, KO, MR], BF16, tag="cn", name=f"cn_{m}")
            bv = bias_pp[bvname]
            for dvc in range(KO):
                nc.any.tensor_mul(ctxn[:, dvc], ctx_my[:, dvc],
                                  bc4[:, dvc // 2])
                nc.vector.tensor_scalar_add(ctxn[:, dvc], ctxn[:, dvc],
                                            bv[:, dvc:dvc + 1])
            return ctxn

        # ---------- stage 0: inputs (DMA priority order) ----------
        xt_full = bigs.tile([P, KO, E], BF16, tag="xtf", name="xt_full")
        nc.sync.dma_start(xt_full, dram["xt_full"].rearrange(
            "(ko p) r -> p ko r", p=P))
        w_qtl = load_w("wfq_tl")

        def load_my(name):
            t = acts.tile([P, KO, MR], BF16, tag="raw", bufs=2,
                          name=f"raw_{name}")
            nc.sync.dma_start(t, dram[name].rearrange("(ko p) r -> p ko r",
                                                      p=P))
            return t

        xl_my = load_my("xl_my")
        xg_my = load_my("xg_my")
        for nm in ("pos_l", "pos_g", "bfq_tl", "bk_tl", "bv_tl", "bfq_tg"):
            bias_pp[nm] = load_bias_pp(nm)

        # ---------- tl MHA front ----------
        inv_text = stats_inv(xt_full)
        bc_text = bcast_row(inv_text, E, dt=BF16)
        qp_tl = qps.tile([P, KO, E], BF16, tag="qp", name="qp_tl")
        gemm(w_qtl, xt_full, qp_tl, bias=bias_pp["bfq_tl"],
             postscale=bc_text)
        kvl = acts.tile([P, KO, MR], BF16, tag="kvl", name="kvl")
        normalize(xl_my, kvl, pos=bias_pp["pos_l"])
        kvg = acts.tile([P, KO, MR], BF16, tag="kvg", name="kvg")
        normalize(xg_my, kvg, pos=bias_pp["pos_g"])
        kp_tl, vp_tl = kv_project("tl", kvl)
        rsin_tl, rsout_tl = reduce_scatter("tl")
        attention_partial("tl", kp_tl, vp_tl, qp_tl, rsin_tl)
        rs_launch(rsin_tl, rsout_tl)

        # remaining biases + own-row text slice (off the critical DMA path)
        for nm in ("bfq_rt", "b_tl", "b_rep", "bq_ff", "bk_tg", "bv_tg",
                   "bfk_ff", "bvh_ff", "bkt_rt", "bvh_rt",
                   "bo_tl", "bo_rt"):
            bias_pp[nm] = load_bias_pp(nm)
        xt_my = load_my("xt_my")
        textn_my = acts.tile([P, KO, MR], BF16, tag="tnm", name="textn_my")
        normalize(xt_my, textn_my)

        # ---------- tg MHA ----------
        qp_tg = qps.tile([P, KO, E], BF16, tag="qp", name="qp_tg")
        gemm(load_w("wfq_tg"), xt_full, qp_tg, bias=bias_pp["bfq_tg"],
             postscale=bc_text)
        kp_tg, vp_tg = kv_project("tg", kvg)
        rsin_tg, rsout_tg = reduce_scatter("tg")
        attention_partial("tg", kp_tg, vp_tg, qp_tg, rsin_tg)
        rs_launch(rsin_tg, rsout_tg)

        # ---------- finish tl -> lt; qp_ff own slice; AllGather ----------
        t_l = acts.tile([P, KO, MR], BF16, tag="t_l", name="t_l")
        gemm(load_w("w_tl"), textn_my, t_l, bias=bias_pp["b_tl"])
        ctxn_tl = ctx_normalize("tl", rsout_tl, "bv_tl")
        lt = acts.tile([P, KO, MR], BF16, tag="lt", name="lt")
        gemm(load_w("wo_tl"), ctxn_tl, lt, bias=bias_pp["bo_tl"],
             residual=t_l)
        qpff_my = acts.tile([P, KO, MR], FP8, tag="qfm", name="qpff_my")
        gemm(load_w("wq_ff"), lt, qpff_my, bias=bias_pp["bq_ff"])
        ag_in = dram_p.tile([CTX_ELEMS], FP8, name="ag_in")
        ag_out = dram_p.tile([8, CTX_ELEMS], FP8, name="ag_out")
        nc.sync.dma_start(ag_in.rearrange("(p a b) -> p a b", p=P, a=KO),
                          qpff_my)
        nc.gpsimd.collective_compute(
            "AllGather", mybir.AluOpType.bypass,
            replica_groups=ALLR,
            ins=[ag_in.opt()], outs=[ag_out.opt()])

        # text-side parts of the fused ff/rt K/V (fill the AG window)
        ktext_ff = acts.tile([P, KO, MR], BF16, tag="ktf", name="ktext_ff")
        gemm(load_w("wbk_ff"), textn_my, ktext_ff, bias=bias_pp["bfk_ff"])
        vtext_ff = kvs.tile([P, 2, 512], BF16, tag="vtx", name="vtext_ff")
        gemm_sm(load_w("wbv_ff"), textn_my, vtext_ff)
        ktext_rt = acts.tile([P, KO, MR], BF16, tag="t_l", name="ktext_rt")
        gemm(load_w("wk_rt"), lt, ktext_rt, bias=bias_pp["bkt_rt"])
        vtext_rt = kvs.tile([P, 2, 512], BF16, tag="vtr", name="vtext_rt")
        gemm_sm(load_w("wv_rt"), lt, vtext_rt)
        qp_rt = qps.tile([P, KO, E], BF16, tag="qp", name="qp_rt")
        gemm(load_w("wfq_rt"), xt_full, qp_rt, bias=bias_pp["bfq_rt"],
             postscale=bc_text)

        # local group columns for the final cosine stage (off critical path)
        xlg = bigs.tile([P, KO, 256], BF16, tag="xlg", name="xlg")
        nc.sync.dma_start(xlg, dram["xlg"].rearrange("(ko p) r -> p ko r",
                                                     p=P))
        inv_lg = stats_inv(xlg, with_eps=True)
        bc_lg = bcast_row(inv_lg, 256, dt=BF16)
        lgn = bigs.tile([P, KO, 256], BF16, tag="lgn", name="lgn")
        for ko in range(KO):
            nc.any.tensor_mul(lgn[:, ko], xlg[:, ko], bc_lg)

        # ---------- ff MHA (kv fused from ctxn_tg, gt never built) -------
        ctxn_tg = ctx_normalize("tg", rsout_tg, "bv_tg")
        kp_ff, vp_ff = kv_fused("ff", ctxn_tg, ktext_ff, vtext_ff, kpdt=FP8)
        qp_ff = qps.tile([P, KO, E], FP8, tag="qp8", bufs=1, name="qp_ff")
        for r in range(8):
            nc.sync.dma_start(qp_ff[:, :, r * MR:(r + 1) * MR],
                              ag_out[r].rearrange("(p a b) -> p a b",
                                                  p=P, a=KO))
        rsin_ff, rsout_ff = reduce_scatter("ff")
        attention_partial("ff", kp_ff, vp_ff, qp_ff, rsin_ff,
                          sc=0.0625 / (FSC * FSC))
        rs_launch(rsin_ff, rsout_ff)

        # ---------- rt MHA (kv fused from ctxn_ff, ff never built) -------
        ctxn_ff = ctx_normalize("ff", rsout_ff, "bvh_ff")
        kp_rt, vp_rt = kv_fused("rt", ctxn_ff, ktext_rt, vtext_rt)
        rsin_rt, rsout_rt = reduce_scatter("rt")
        attention_partial("rt", kp_rt, vp_rt, qp_rt, rsin_rt)
        rs_launch(rsin_rt, rsout_rt)
        # t_r fills the PE window while RS_rt is in flight
        t_r = qps.tile([P, KO, E], BF16, tag="qp", name="t_r")
        gemm(load_w("w_rep"), xt_full, t_r, bias=bias_pp["b_rep"],
             postscale=bc_text)
        ctxn_rt = ctx_normalize("rt", rsout_rt, "bvh_rt")
        rt = acts.tile([P, KO, MR], BF16, tag="rt", name="rt")
        gemm(load_w("wo_rt"), ctxn_rt, rt, bias=bias_pp["bo_rt"])

        # ---------- full = rt @ t_r.T (fullT [j, own i]), cosine ----------
        fullT = bigs.tile([P, KO, MR], F32, tag="flt", name="fullT")
        for jc in range(KO):
            ps = psB.tile([P, MR], F32, tag="mmb", name="ps_full")
            for ko in range(KO):
                nc.tensor.matmul(ps, t_r[:, ko, jc * P:(jc + 1) * P],
                                 rt[:, ko], start=(ko == 0),
                                 stop=(ko == KO - 1))
            nc.any.tensor_copy(out=fullT[:, jc], in_=ps)
        inv_full = stats_inv(fullT, sqdt=F32R, with_eps=True)
        bc_full = bcast_row(inv_full, MR, dt=F32)
        ffn = bigs.tile([P, KO, MR], BF16, tag="ffn", name="ffn")
        for jc in range(KO):
            nc.any.tensor_mul(ffn[:, jc], fullT[:, jc], bc_full)

        lg = outs.tile([P, 2, MR], F32, name="lg")
        for kc in range(2):
            ps = psB.tile([P, MR], F32, tag="mmb", name="ps_lg")
            for jc in range(KO):
                nc.tensor.matmul(ps, lgn[:, jc, kc * P:(kc + 1) * P],
                                 ffn[:, jc], start=(jc == 0),
                                 stop=(jc == KO - 1))
            nc.any.tensor_copy(out=lg[:, kc], in_=ps)
        nc.sync.dma_start(out_logits.rearrange("(kc p) i -> p kc i", p=P), lg)

    nc.compile()
    return nc


def make_in_maps(local_feat, global_feat, text_feat,
                 w_tl, b_tl, w_tg, b_tg, w_rep, b_rep,
                 pos_local, pos_global, mha_params):
    f32 = np.float32
    bf16 = ml_dtypes.bfloat16

    def tb(a):
        return np.ascontiguousarray(a.T.astype(bf16))

    textT = np.ascontiguousarray(text_feat.T.astype(bf16))
    localT = np.ascontiguousarray(local_feat.T.astype(f32))
    globT = np.ascontiguousarray(global_feat.T.astype(bf16))

    tl_wi, tl_bi, tl_wo, tl_bo = mha_params["tl"]
    tg_wi, tg_bi, tg_wo, tg_bo = mha_params["tg"]
    ff_wi, ff_bi, ff_wo, ff_bo = mha_params["ff"]
    rt_wi, rt_bi, rt_wo, rt_bo = mha_params["rt"]
    S = FSC_HOST

    shared = {
        "xt_full": textT,
        "w_tl": tb(w_tl), "w_rep": tb(w_rep),
        "b_tl": b_tl.astype(f32), "b_rep": b_rep.astype(f32),
        "pos_l": pos_local.astype(f32), "pos_g": pos_global.astype(f32),
    }
    # fused text->qp weights for the three text-sourced MHAs
    for m, wx, bx in (("tl", w_tl, b_tl), ("tg", w_tg, b_tg),
                      ("rt", w_rep, b_rep)):
        wi, bi, _, _ = mha_params[m]
        wq, bq = wi[:E], bi[:E]
        shared[f"wfq_{m}"] = tb(wq @ wx)
        shared[f"bfq_{m}"] = (wq @ bx + bq).astype(f32)
    # tl / tg raw K/V weights
    for m in ("tl", "tg"):
        wi, bi, _, _ = mha_params[m]
        shared[f"wk_{m}"] = tb(wi[E:2 * E])
        shared[f"wv_{m}"] = tb(wi[2 * E:])
        shared[f"bk_{m}"] = bi[E:2 * E].astype(f32)
        shared[f"bv_{m}"] = bi[2 * E:].astype(f32)
    shared["wo_tl"] = tb(tl_wo)
    shared["bo_tl"] = tl_bo.astype(f32)
    shared["wo_rt"] = tb(rt_wo)
    shared["bo_rt"] = rt_bo.astype(f32)
    # ff q path (fp8, pre-scaled)
    shared["wq_ff"] = tb(ff_wi[:E] * S)
    shared["bq_ff"] = (ff_bi[:E] * S).astype(f32)
    # ff K/V fused through tg's out-projection (gt never materialized):
    # kp_ff = S*(ctxn_tg @ (Wk Wo_tg)^T + textn @ (Wk W_tg)^T
    #            + Wk@(bo_tg + b_tg) + bk)
    wk_ff, wv_ff = ff_wi[E:2 * E], ff_wi[2 * E:]
    bk_ff, bv_ff = ff_bi[E:2 * E], ff_bi[2 * E:]
    shared["wak_ff"] = tb(wk_ff @ tg_wo * S)
    shared["wbk_ff"] = tb(wk_ff @ w_tg * S)
    shared["bfk_ff"] = (S * (wk_ff @ (tg_bo + b_tg) + bk_ff)).astype(f32)
    shared["wav_ff"] = tb(wv_ff @ tg_wo)
    shared["wbv_ff"] = tb(wv_ff @ w_tg)
    shared["bvh_ff"] = (wv_ff @ (tg_bo + b_tg) + bv_ff).astype(f32)
    # rt K/V fused through ff's out-projection (ff never materialized):
    # kp_rt = ctxn_ff @ (Wk_rt Wo_ff)^T + lt @ Wk_rt^T + Wk_rt@bo_ff + bk_rt
    wk_rt, wv_rt = rt_wi[E:2 * E], rt_wi[2 * E:]
    bk_rt, bv_rt = rt_bi[E:2 * E], rt_bi[2 * E:]
    shared["wak_rt"] = tb(wk_rt @ ff_wo)
    shared["wk_rt"] = tb(wk_rt)
    shared["bkt_rt"] = (wk_rt @ ff_bo + bk_rt).astype(f32)
    shared["wav_rt"] = tb(wv_rt @ ff_wo)
    shared["wv_rt"] = tb(wv_rt)
    shared["bvh_rt"] = (wv_rt @ ff_bo + bv_rt).astype(f32)

    in_maps = []
    for c in range(NCORES):
        rs = slice(MR * c, MR * (c + 1))
        g = c // 2
        m = dict(shared)
        m["xt_my"] = np.ascontiguousarray(textT[:, rs])
        m["xl_my"] = np.ascontiguousarray(localT[:, rs].astype(bf16))
        m["xg_my"] = np.ascontiguousarray(globT[:, rs])
        m["xlg"] = np.ascontiguousarray(
            localT[:, 256 * g:256 * (g + 1)].astype(bf16))
        in_maps.append(m)
    return in_maps


FSC_HOST = FSC


def kernel(local_feat, global_feat, text_feat,
           w_tl, b_tl, w_tg, b_tg, w_rep, b_rep,
           pos_local, pos_global,
           tl_wi, tl_bi, tl_wo, tl_bo,
           tg_wi, tg_bi, tg_wo, tg_bo,
           ff_wi, ff_bi, ff_wo, ff_bo,
           rt_wi, rt_bi, rt_wo, rt_bo,
           n_groups):
    assert int(n_groups) == 4
    if "nc" not in _CACHE:
        _CACHE["nc"] = build_nc()
    nc = _CACHE["nc"]
    mha_params = {
        "tl": (tl_wi, tl_bi, tl_wo, tl_bo),
        "tg": (tg_wi, tg_bi, tg_wo, tg_bo),
        "ff": (ff_wi, ff_bi, ff_wo, ff_bo),
        "rt": (rt_wi, rt_bi, rt_wo, rt_bo),
    }
    in_maps = make_in_maps(np.asarray(local_feat), np.asarray(global_feat),
                           np.asarray(text_feat),
                           np.asarray(w_tl), np.asarray(b_tl),
                           np.asarray(w_tg), np.asarray(b_tg),
                           np.asarray(w_rep), np.asarray(b_rep),
                           np.asarray(pos_local), np.asarray(pos_global),
                           {k: tuple(np.asarray(x) for x in v)
                            for k, v in mha_params.items()})
    res = run_bass_kernel_spmd(nc, in_maps, core_ids=list(range(NCORES)))
    _CACHE["last_results"] = res
    out = np.empty((4, 256, 256), dtype=np.float32)
    for c in range(NCORES):
        g = c // 2
        i0 = MR * (c % 2)
        out[g][i0:i0 + MR, :] = res.results[c]["logits"].T
    return out

